# revision 1
# baseline (speedup 1.0000x reference)
"""MoE (16 experts, top-1 gate, D=H=768) Trainium2 kernel.

Strategy (expert-parallel, per the sharding hint):
  - Host computes the gate (logits argmax) — this IS the dispatch step that
    decides the sharding: tokens are routed to the core owning their expert.
  - 16 experts are sharded 2-per-core across the 8 NeuronCores. Experts are
    sorted by routed-token count: the 8 largest go in slot 0 (capacity C0),
    the 8 smallest in slot 1 (capacity C1 <= C0), so every core does the
    identical padded work and padding waste is minimized.
  - Each core runs the two-GEMM MLP (x @ W1.T -> GELU -> @ W2.T) for its two
    experts over its routed tokens, padded to the slot capacity.
  - Host scatters per-token outputs back to the full [B, N, D] tensor.

Device kernel details:
  - Matmul operands are fp16 (PE full rate + FWL weight loads; fp32
    LDWEIGHTS cannot pipeline and halves matmul throughput; fp16 has 10
    mantissa bits -> rel err ~4e-4 end to end). PSUM accumulation is fp32,
    biases/GELU applied on fp32 PSUM, outputs fp32. MM_DTYPE switches to
    "bf16" (~3e-3) or "f32r" (~2e-4 but ~40% slower).
  - Layouts are pre-transposed on host so the device only does contiguous
    DMAs: the first GEMM computes H^T = W1 @ x^T accumulating over six
    128-row d-chunks; GELU(+b1) is applied PSUM->SBUF on the scalar engine;
    the second GEMM computes Y^T = W2 @ G the same way, then a per-partition
    b2 add on the vector engine, then DMA out.
  - All input DMAs are issued before any compute-dependent instruction:
    weights ride the SP HWDGE ring, x/biases (and outputs, later) the ACT
    ring, so both rings stream continuously. The first expert's w1/x arrive
    in progressively larger pieces so MM1 starts as early as possible.
"""

import json

import ml_dtypes
import numpy as np

import concourse.bass as bass
import concourse.mybir as mybir
import concourse.tile as tile
from concourse.bass_utils import run_bass_kernel_spmd

E = 16          # experts
D = 768         # d_model
H = 768         # d_hidden
NCORES = 8
EPC = E // NCORES   # experts (slots) per core = 2
DC = D // 128       # 6 d-chunks
HC = H // 128       # 6 h-chunks

MM_DTYPE = "f16"   # "f16" | "bf16" | "f32r"

F32 = mybir.dt.float32


def _mm_dt():
    if MM_DTYPE == "f16":
        # fp16 runs at the same PE rate as bf16 (1 col/cycle + FWL weight
        # loads) but has 10 mantissa bits instead of 7 — ~6x lower rounding
        # error. All operands here (|x| < ~6, |W| < ~0.2, GELU outputs) are
        # far inside fp16 range and accumulation is fp32 PSUM.
        return mybir.dt.float16, np.float16
    if MM_DTYPE == "bf16":
        return mybir.dt.bfloat16, ml_dtypes.bfloat16
    return mybir.dt.float32r, np.float32


def _split_multi_waits(nc):
    """Walrus (this image's build) rejects >1 sem-wait on one instruction
    ("Too many sync wait commands" on the TileContext-exit Drain). Move
    excess waits onto a chain of same-engine NoOps directly before the
    instruction — the sequencer runs them in program order, so the
    happens-after relation is preserved exactly."""
    bir = json.loads(nc.to_json_bytes())
    nid = 0
    for fn in bir["functions"]:
        for blk in fn["blocks"]:
            out = []
            for ins in blk["instructions"]:
                si = ins.get("sync_info")
                waits = (si or {}).get("on_wait") or []
                if len(waits) > 1:
                    for w in waits[:-1]:
                        nid += 1
                        out.append({
                            "debug": ins.get("debug", 0),
                            "name": f"I-waitfix{nid}",
                            "opcode": "NoOp",
                            "engine": ins["engine"],
                            "ins": [],
                            "outs": [],
                            "sync_info": {"on_update": [], "on_wait": [w]},
                        })
                    si["on_wait"] = waits[-1:]
                out.append(ins)
            blk["instructions"] = out
    data = json.dumps(bir).encode()
    nc.to_json_bytes = lambda: data
    return nc


def _chunking(C):
    chunks = []
    c0 = 0
    while c0 < C:
        cw = min(512, C - c0)
        chunks.append((c0, cw))
        c0 += cw
    return chunks


def _build(C0, C1):
    """Per-core SPMD kernel: slot 0 with token capacity C0, slot 1 with C1
    (both multiples of 128, >=256). Token dim in chunks of <=512 (PSUM bank
    limit for fp32 accumulation)."""
    caps = [C0, C1]
    slot_chunks = [_chunking(C) for C in caps]

    MMDT, _ = _mm_dt()

    nc = bass.Bass("TRN2", target_bir_lowering=False, debug=False,
                   num_devices=NCORES)
    # Layouts match the SBUF tiles exactly (partition-major) so every DMA is
    # a large contiguous burst.
    xts_d = [nc.dram_tensor(f"xt{s}", [128, DC, caps[s]], MMDT,
                            kind="ExternalInput") for s in range(EPC)]
    yts_d = [nc.dram_tensor(f"yt{s}", [128, DC, caps[s]], F32,
                            kind="ExternalOutput") for s in range(EPC)]
    w1t = nc.dram_tensor("w1t", [EPC, 128, DC, H], MMDT, kind="ExternalInput")
    w2t = nc.dram_tensor("w2t", [EPC, 128, HC, D], MMDT, kind="ExternalInput")
    b1c = nc.dram_tensor("b1c", [EPC, 128, HC], F32, kind="ExternalInput")
    b2c = nc.dram_tensor("b2c", [EPC, 128, DC], F32, kind="ExternalInput")

    GELU = mybir.ActivationFunctionType.Gelu

    with tile.TileContext(nc) as tc:
        with (
            # x/w tiles have unique tags (one tile per tag, alive for the
            # whole kernel) -> bufs=1 keeps SBUF usage exact even when C is
            # large (heavily skewed routing).
            tc.tile_pool(name="xp", bufs=1) as xp,
            tc.tile_pool(name="wp", bufs=1) as wp,
            tc.tile_pool(name="gp", bufs=2) as gp,
            tc.tile_pool(name="yp", bufs=3) as yp,
            tc.tile_pool(name="bp", bufs=2) as bp,
            tc.tile_pool(name="pp", bufs=4, space="PSUM") as pp,
        ):
            # ---- phase 1: issue ALL input DMAs. No compute-dependent wait
            # ever enters either HWDGE ring, so both stream continuously.
            tiles = []
            for s in range(EPC):
                chunks = slot_chunks[s]
                w1s = wp.tile([128, DC, H], MMDT, tag=f"w1_{s}",
                              name=f"w1s_{s}")
                w2s = wp.tile([128, HC, D], MMDT, tag=f"w2_{s}",
                              name=f"w2s_{s}")
                xcs = [xp.tile([128, DC, 512], MMDT, tag=f"x_{s}_{ci}",
                               name=f"xc_{s}_{ci}")
                       for ci in range(len(chunks))]
                if s == 0:
                    # progressive pieces: MM1 can start after ~0.3 MB
                    for dl, dh in ((0, 1), (1, 3), (3, 6)):
                        nc.sync.dma_start(w1s[:, dl:dh],
                                          w1t.ap()[s, :, dl:dh])
                        for ci, (c0, cw) in enumerate(chunks):
                            nc.scalar.dma_start(
                                xcs[ci][:, dl:dh, :cw],
                                xts_d[s].ap()[:, dl:dh, c0:c0 + cw])
                else:
                    nc.sync.dma_start(w1s[:, :, :], w1t.ap()[s])
                    for ci, (c0, cw) in enumerate(chunks):
                        nc.scalar.dma_start(xcs[ci][:, :, :cw],
                                            xts_d[s].ap()[:, :, c0:c0 + cw])
                # balance the rings: slot1's w2 rides the ACT ring (which
                # finishes x early), so slot1's w1 lands sooner on SP.
                if s == 0:
                    nc.sync.dma_start(w2s[:, :, :], w2t.ap()[s])
                else:
                    nc.scalar.dma_start(w2s[:, :, :], w2t.ap()[s])
                b1s = bp.tile([128, HC], F32, tag="b1", name=f"b1s_{s}")
                nc.scalar.dma_start(b1s[:, :], b1c.ap()[s])
                b2s = bp.tile([128, DC], F32, tag="b2", name=f"b2s_{s}")
                nc.scalar.dma_start(b2s[:, :], b2c.ap()[s])
                tiles.append((w1s, w2s, xcs, b1s, b2s))

            # ---- phase 2: compute
            for s in range(EPC):
                chunks = slot_chunks[s]
                w1s, w2s, xcs, b1s, b2s = tiles[s]
                last_slot = (s == EPC - 1)
                for ci, (c0, cw) in enumerate(chunks):
                    xc = xcs[ci]
                    last_chunk = last_slot and (ci == len(chunks) - 1)
                    gc = gp.tile([128, HC, 512], MMDT, tag="g")
                    for hc in range(HC):
                        ps = pp.tile([128, 512], F32, tag="ps")
                        for dc in range(DC):
                            nc.tensor.matmul(
                                ps[:, :cw],
                                w1s[:, dc, hc * 128:(hc + 1) * 128],
                                xc[:, dc, :cw],
                                start=(dc == 0), stop=(dc == DC - 1),
                            )
                        nc.scalar.activation(gc[:, hc, :cw], ps[:, :cw], GELU,
                                             bias=b1s[:, hc:hc + 1], scale=1.0)
                    # second GEMM; outputs grouped 3 d-chunks per DMA for
                    # bandwidth, except the very last group which flushes
                    # per-d-chunk so the tail pipeline drains early.
                    for g2 in range(2):
                        dl, dh = 3 * g2, 3 * (g2 + 1)
                        split_out = last_chunk and g2 == 1
                        yc = yp.tile([128, 3, 512], F32, tag="y",
                                     name=f"yc_{s}_{ci}_{g2}")
                        for dc in range(dl, dh):
                            ps2 = pp.tile([128, 512], F32, tag="ps")
                            for hc in range(HC):
                                nc.tensor.matmul(
                                    ps2[:, :cw],
                                    w2s[:, hc, dc * 128:(dc + 1) * 128],
                                    gc[:, hc, :cw],
                                    start=(hc == 0), stop=(hc == HC - 1),
                                )
                            nc.vector.tensor_scalar_add(
                                yc[:, dc - dl, :cw], ps2[:, :cw],
                                b2s[:, dc:dc + 1])
                            if split_out:
                                nc.scalar.dma_start(
                                    yts_d[s].ap()[:, dc, c0:c0 + cw],
                                    yc[:, dc - dl, :cw])
                        if not split_out:
                            nc.scalar.dma_start(
                                yts_d[s].ap()[:, dl:dh, c0:c0 + cw],
                                yc[:, :, :cw])

    return _split_multi_waits(nc)


_NC_CACHE = {}


def _get_nc(C0, C1):
    key = (C0, C1, MM_DTYPE)
    nc = _NC_CACHE.get(key)
    if nc is None:
        nc = _build(C0, C1)
        _NC_CACHE[key] = nc
    return nc


def _cap(n):
    return int(max(256, -(-int(n) // 128) * 128))


def kernel(x, W1, b1, W2, b2, Wg, bg):
    x = np.ascontiguousarray(np.asarray(x, dtype=np.float32))
    W1 = np.asarray(W1, dtype=np.float32)
    b1 = np.asarray(b1, dtype=np.float32)
    W2 = np.asarray(W2, dtype=np.float32)
    b2 = np.asarray(b2, dtype=np.float32)
    Wg = np.asarray(Wg, dtype=np.float32)
    bg = np.asarray(bg, dtype=np.float32)

    B, N, Dx = x.shape
    assert Dx == D and W1.shape == (E, H, D)
    T = B * N
    t = x.reshape(T, D)

    # --- gate / dispatch (host): this decides the sharding ---
    logits = t @ Wg.T + bg
    idx = np.argmax(logits, axis=1)

    counts = np.bincount(idx, minlength=E)
    # slot 0 <- 8 largest experts, slot 1 <- 8 smallest
    order = np.argsort(-counts, kind="stable")
    slot_experts = [order[:NCORES], order[NCORES:]]
    C0 = _cap(counts[slot_experts[0]].max())
    C1 = _cap(counts[slot_experts[1]].max())
    caps = [C0, C1]
    nc = _get_nc(C0, C1)
    _, npdt = _mm_dt()

    tok_ids = [np.nonzero(idx == e)[0] for e in range(E)]

    # --- host-side layout prep ---
    t_mm = t.astype(npdt)
    # w1t[e, i, dc, h] = W1[e, h, dc*128+i] (partition-major, chunk, col)
    w1t_all = np.ascontiguousarray(
        W1.astype(npdt).transpose(0, 2, 1).reshape(E, DC, 128, H)
        .transpose(0, 2, 1, 3))
    w2t_all = np.ascontiguousarray(
        W2.astype(npdt).transpose(0, 2, 1).reshape(E, HC, 128, D)
        .transpose(0, 2, 1, 3))
    # b1c[e, i, hc] = b1[e, hc*128+i]
    b1c_all = np.ascontiguousarray(b1.reshape(E, HC, 128).transpose(0, 2, 1))
    b2c_all = np.ascontiguousarray(b2.reshape(E, DC, 128).transpose(0, 2, 1))

    in_maps = []
    for c in range(NCORES):
        experts = [int(slot_experts[s][c]) for s in range(EPC)]
        m = {
            "w1t": np.ascontiguousarray(w1t_all[experts]),
            "w2t": np.ascontiguousarray(w2t_all[experts]),
            "b1c": np.ascontiguousarray(b1c_all[experts]),
            "b2c": np.ascontiguousarray(b2c_all[experts]),
        }
        for s in range(EPC):
            C = caps[s]
            xts = np.zeros((128, DC, C), npdt)
            ids = tok_ids[experts[s]]
            n = len(ids)
            if n:
                xts[:, :, :n] = (
                    t_mm[ids].T.reshape(DC, 128, n).transpose(1, 0, 2))
            m[f"xt{s}"] = xts
        in_maps.append(m)

    res = run_bass_kernel_spmd(nc, in_maps, core_ids=list(range(NCORES)))

    out = np.empty((T, D), np.float32)
    for c in range(NCORES):
        for s in range(EPC):
            e = int(slot_experts[s][c])
            ids = tok_ids[e]
            n = len(ids)
            if n:
                yt = res.results[c][f"yt{s}"]  # [128, DC, C]
                out[ids] = yt.transpose(1, 0, 2).reshape(D, caps[s])[:, :n].T
    return out.reshape(B, N, D)

